# revision 3
# baseline (speedup 1.0000x reference)
"""Batch Child-Sum TreeLSTM cell on 8 Trainium2 NeuronCores.

Strategy (data-parallel over nodes; fp16 matmuls, PE-roofline ~195us/core):
  - Shard the N nodes (and their contiguous child segments) evenly across the
    8 cores; replicate the small weight matrices. Irregular sorted
    segment_ids are first regularized host-side by zero-padding every node to
    max_children slots (exact: padded slots contribute 0).
  - Host stages activations feature-major (features on SBUF partitions) and
    child-major (one contiguous slab per child slot), cast to fp16 (same PE
    rate as bf16, 10-bit mantissa - strictly more accurate for this O(5)
    data; fp8 was measured too coarse: ~3e-2 rel err vs the 2e-2 gate).
  - PE: 18 matmul streams per node (z: 3 chunks x (2 x-halves + h_tilde);
    f: 3 slots x (U_f h_k + 2 W_f-halves)), all [128,256]-wide into PSUM.
  - PSUM: per 256-node sub, one 5-region tile [z_i|z_o|f0|f1|f2] plus a
    separate z_u tile, both double-buffered (6+2 half-banks of 8) so the PE
    never waits on ACT evacuation (the old kernel single-buffered the f
    PSUM, serializing PE behind ACT every sub-tile).
  - ACT (the old second bottleneck at ~192us) drops to ~176us: ONE fused
    sigmoid evacuates z_i|z_o|f0|f1|f2 per sub (biases are zero per the
    input spec; a general per-region-bias path is kept for nonzero biases),
    tanh(z_u) per sub, tanh(c) once per 2560-node macro.
  - DVE (old ~181us) drops to ~140us: the whole gate chain runs ONCE per
    macro as 9 wide fp16 2x-mode tensor ops (h_tilde child-sum, f*c, segment
    sums, c/h assembly) using strided 3D access patterns over the per-sub
    sigmoid outputs - per-op overhead (~100 cyc) amortizes over 2560 cols.
  - Outputs store as fp16 on the idle gpsimd SWDGE queue; host upcasts.
    Engine budgets/core: PE ~195us (bound), DMA 64MB ~170us, ACT ~176us,
    DVE ~140us.
"""

from contextlib import ExitStack

import numpy as np

import concourse.bass as bass
import concourse.bacc as bacc
import concourse.tile as tile
from concourse import mybir
from concourse.bass_utils import run_bass_kernel_spmd

F32 = mybir.dt.float32
FP16 = mybir.dt.float16

N_CORES = 8

# Tiling (in nodes). SUB: PSUM region width (5 regions x 256 f32 = 2.5 banks
# + z_u half-bank, double-buffered). MACRO: DMA / SBUF / DVE-chain
# granularity.
SUB = 256
MACRO = 2560


def _chunks(total, step):
    out = []
    off = 0
    while off < total:
        out.append((off, min(step, total - off)))
        off += step
    return out


def build_program(npc, in_dim, hid, cpn, zero_bias=True):
    """Bass program for one core's shard: npc nodes, npc*cpn edges."""
    assert in_dim == 256 and hid == 128
    assert npc % 512 == 0
    assert 1 <= cpn <= 4, "PSUM layout fits at most 4 child slots"
    NR = 2 + cpn  # A-tile regions per sub: si, so, f0..f{cpn-1}
    NZW = NR * SUB
    FOFF = 2 * SUB  # f-region offset inside A

    nc = bacc.Bacc("TRN2", target_bir_lowering=False, debug=False)

    xT = nc.dram_tensor("xT", [hid, 2 * npc], FP16, kind="ExternalInput").ap()
    ch = nc.dram_tensor("ch", [hid, cpn * npc], FP16, kind="ExternalInput").ap()
    cc = nc.dram_tensor("cc", [hid, cpn * npc], FP16, kind="ExternalInput").ap()
    wcx = nc.dram_tensor("wcx", [hid, 2 * 3 * hid], FP16, kind="ExternalInput").ap()
    wch = nc.dram_tensor("wch", [hid, 3 * hid], FP16, kind="ExternalInput").ap()
    wfd = nc.dram_tensor("wfd", [hid, 2 * hid], FP16, kind="ExternalInput").ap()
    uf = nc.dram_tensor("uf", [hid, hid], FP16, kind="ExternalInput").ap()
    bc3 = nc.dram_tensor("bc3", [hid, 3], F32, kind="ExternalInput").ap()
    bf1 = nc.dram_tensor("bf1", [hid, 1], F32, kind="ExternalInput").ap()

    cT = nc.dram_tensor("cT", [hid, npc], FP16, kind="ExternalOutput").ap()
    hT = nc.dram_tensor("hT", [hid, npc], FP16, kind="ExternalOutput").ap()

    xT3 = xT.rearrange("p (i n) -> p i n", i=2)
    ch3 = ch.rearrange("p (c n) -> p c n", c=cpn)
    cc3 = cc.rearrange("p (c n) -> p c n", c=cpn)

    ACTF = mybir.ActivationFunctionType

    with tile.TileContext(nc) as tc, ExitStack() as ctx:
        consts = ctx.enter_context(tc.tile_pool(name="consts", bufs=1))
        macro_pool = ctx.enter_context(tc.tile_pool(name="macro", bufs=2))
        tail_pool = ctx.enter_context(tc.tile_pool(name="tail", bufs=1))
        psum = ctx.enter_context(tc.tile_pool(name="psum", bufs=2, space="PSUM"))

        # ---- weights (resident) ----
        wcx_sb = consts.tile([128, 2 * 3 * hid], FP16, tag="wcx")
        nc.sync.dma_start(out=wcx_sb, in_=wcx)
        wcx3 = wcx_sb.rearrange("p (i m) -> p i m", i=2)
        wch_sb = consts.tile([128, 3 * hid], FP16, tag="wch")
        nc.sync.dma_start(out=wch_sb, in_=wch)
        wfd_sb = consts.tile([128, 2 * hid], FP16, tag="wfd")
        nc.sync.dma_start(out=wfd_sb, in_=wfd)
        wfd3 = wfd_sb.rearrange("p (i m) -> p i m", i=2)
        uf_sb = consts.tile([128, hid], FP16, tag="uf")
        nc.sync.dma_start(out=uf_sb, in_=uf)
        bc_sb = consts.tile([128, 3], F32, tag="bc3")
        nc.sync.dma_start(out=bc_sb, in_=bc3)
        bf_sb = consts.tile([128, 1], F32, tag="bf1")
        nc.sync.dma_start(out=bf_sb, in_=bf1)

        macro_plan = []
        if npc > 2 * MACRO:
            # small first macro (compute starts early) and small last macro
            # (stores start early, short tail)
            macro_plan.append((0, 512))
            macro_plan += [(512 + o, s) for o, s in _chunks(npc - 1024, MACRO)]
            macro_plan.append((npc - 512, 512))
        else:
            macro_plan = _chunks(npc, MACRO)

        for m0, msz in macro_plan:
            nsub = msz // SUB
            x_t = macro_pool.tile([128, 2 * msz], FP16, tag="x")
            nc.sync.dma_start(
                out=x_t.rearrange("p (i n) -> p i n", i=2),
                in_=xT3[:, :, m0 : m0 + msz],
            )
            xt3 = x_t.rearrange("p (i n) -> p i n", i=2)
            ch_t = macro_pool.tile([128, cpn * msz], FP16, tag="ch")
            nc.sync.dma_start(
                out=ch_t.rearrange("p (c n) -> p c n", c=cpn),
                in_=ch3[:, :, m0 : m0 + msz],
            )
            cht3 = ch_t.rearrange("p (c n) -> p c n", c=cpn)
            cc_t = macro_pool.tile([128, cpn * msz], FP16, tag="cc")
            nc.sync.dma_start(
                out=cc_t.rearrange("p (c n) -> p c n", c=cpn),
                in_=cc3[:, :, m0 : m0 + msz],
            )
            cct3 = cc_t.rearrange("p (c n) -> p c n", c=cpn)

            # h_tilde for the whole macro: 2 wide DVE adds (PE consumes it)
            ht_t = macro_pool.tile([128, msz], FP16, tag="ht")
            nc.vector.tensor_add(ht_t, cht3[:, 0, :], cht3[:, 1, :])
            for ci in range(2, cpn):
                nc.vector.tensor_add(ht_t, ht_t, cht3[:, ci, :])

            sio_t = macro_pool.tile([128, nsub * NZW], FP16, tag="sio")
            tu_t = macro_pool.tile([128, msz], FP16, tag="tu")

            for k in range(nsub):
                s0 = k * SUB
                A = psum.tile([128, NZW], F32, tag="A")
                zu = psum.tile([128, SUB], F32, tag="zu")
                xs = xt3[:, :, s0 : s0 + SUB]
                hts = ht_t[:, s0 : s0 + SUB]
                # z chunks: j=0 -> si, j=1 -> so, j=2 -> zu tile
                for j in range(3):
                    out = zu if j == 2 else A[:, j * SUB : (j + 1) * SUB]
                    for i in range(2):
                        nc.tensor.matmul(
                            out,
                            lhsT=wcx3[:, i, 128 * j : 128 * (j + 1)],
                            rhs=xs[:, i, :],
                            start=(i == 0),
                            stop=False,
                        )
                    nc.tensor.matmul(
                        out,
                        lhsT=wch_sb[:, 128 * j : 128 * (j + 1)],
                        rhs=hts,
                        start=False,
                        stop=True,
                    )
                # forget gates, child-major: f_c = U_f h_c + W_f x
                for c in range(cpn):
                    out = A[:, FOFF + c * SUB : FOFF + (c + 1) * SUB]
                    nc.tensor.matmul(
                        out,
                        lhsT=uf_sb,
                        rhs=cht3[:, c, s0 : s0 + SUB],
                        start=True,
                        stop=False,
                    )
                    for i in range(2):
                        nc.tensor.matmul(
                            out, lhsT=wfd3[:, i, :], rhs=xs[:, i, :],
                            start=False, stop=(i == 1),
                        )

                sio = sio_t[:, k * NZW : (k + 1) * NZW]
                if zero_bias:
                    # one transcendental pass for all five A regions
                    nc.scalar.activation(sio, A, ACTF.Sigmoid)
                else:
                    nc.scalar.activation(
                        sio[:, 0:SUB], A[:, 0:SUB], ACTF.Sigmoid,
                        bias=bc_sb[:, 0:1],
                    )
                    nc.scalar.activation(
                        sio[:, SUB : 2 * SUB], A[:, SUB : 2 * SUB], ACTF.Sigmoid,
                        bias=bc_sb[:, 1:2],
                    )
                    nc.scalar.activation(
                        sio[:, FOFF:NZW], A[:, FOFF:NZW], ACTF.Sigmoid,
                        bias=bf_sb[:, 0:1],
                    )
                nc.scalar.activation(
                    tu_t[:, s0 : s0 + SUB], zu, ACTF.Tanh,
                    **({} if zero_bias else {"bias": bc_sb[:, 2:3]}),
                )

            # ---- whole-macro gate chain on DVE (wide 2x-mode fp16 ops) ----
            sioM = sio_t.rearrange("p (k w) -> p k w", w=NZW)
            si3 = sioM[:, :, 0:SUB]
            so3 = sioM[:, :, SUB : 2 * SUB]
            c_t = macro_pool.tile([128, msz], FP16, tag="c_out")
            h_t = macro_pool.tile([128, msz], FP16, tag="h_out")
            tc_t = macro_pool.tile([128, msz], FP16, tag="tanh_c")
            fjc_t = tail_pool.tile([128, cpn * msz], FP16, tag="fjc")
            fjc3 = fjc_t.rearrange("p (c n) -> p c n", c=cpn)
            for c in range(cpn):
                fcol = sioM[:, :, FOFF + c * SUB : FOFF + (c + 1) * SUB]
                nc.vector.tensor_mul(
                    fjc3[:, c, :].rearrange("p (k n) -> p k n", n=SUB),
                    fcol,
                    cct3[:, c, :].rearrange("p (k n) -> p k n", n=SUB),
                )
            if cpn == 1:
                fc_t = fjc_t
            else:
                fc_t = tail_pool.tile([128, msz], FP16, tag="fc")
                nc.vector.tensor_add(fc_t, fjc3[:, 0, :], fjc3[:, 1, :])
                for ci in range(2, cpn):
                    nc.vector.tensor_add(fc_t, fc_t, fjc3[:, ci, :])
            c3 = c_t.rearrange("p (k n) -> p k n", n=SUB)
            nc.vector.tensor_mul(c3, si3, tu_t.rearrange("p (k n) -> p k n", n=SUB))
            nc.vector.tensor_add(c_t, c_t, fc_t)
            nc.scalar.activation(tc_t, c_t, ACTF.Tanh)
            nc.vector.tensor_mul(
                h_t.rearrange("p (k n) -> p k n", n=SUB),
                so3,
                tc_t.rearrange("p (k n) -> p k n", n=SUB),
            )
            # stores ride the otherwise-idle gpsimd SWDGE queue so the next
            # macro's loads never queue behind them in the SP HWDGE FIFO
            nc.gpsimd.dma_start(out=cT[:, m0 : m0 + msz], in_=c_t)
            nc.gpsimd.dma_start(out=hT[:, m0 : m0 + msz], in_=h_t)

    nc.compile()
    return nc


TRACE = False  # set True (e.g. from test.py) to capture an NTFF profile
LAST_RESULTS = None  # BassKernelResults of the most recent kernel() call

_PROGRAM_CACHE = {}


def _get_program(npc, in_dim, hid, cpn, zero_bias):
    key = (npc, in_dim, hid, cpn, zero_bias, SUB, MACRO)
    if key not in _PROGRAM_CACHE:
        _PROGRAM_CACHE[key] = build_program(npc, in_dim, hid, cpn, zero_bias)
    return _PROGRAM_CACHE[key]


def _pad_children(child_c, child_h, segment_ids, n):
    """Regularize to exactly max_c children per node (zero padding is exact:
    padded slots contribute sigmoid(..)*0 to fc and 0 to the child sum)."""
    seg = np.asarray(segment_ids).astype(np.int64)
    e = seg.shape[0]
    counts = np.bincount(seg, minlength=n)
    max_c = int(counts.max()) if e else 1
    if e == n * max_c and np.all(counts == max_c):
        return child_c, child_h, max_c  # already regular (and sorted)
    hid = child_h.shape[1]
    slot = np.arange(e, dtype=np.int64) - np.repeat(
        np.concatenate([[0], np.cumsum(counts)[:-1]]), counts
    )
    cc = np.zeros((n * max_c, hid), np.float32)
    ch = np.zeros((n * max_c, hid), np.float32)
    idx = seg * max_c + slot
    cc[idx] = child_c
    ch[idx] = child_h
    return cc, ch, max_c


def _stage_weights(W_combined, W_f, U_f, b_combined, b_f, hid):
    Wc = np.asarray(W_combined, dtype=np.float32)
    wcx = np.ascontiguousarray(
        Wc[: 2 * hid].reshape(2, hid, 3 * hid).transpose(1, 0, 2).astype(np.float16)
    ).reshape(hid, 2 * 3 * hid)
    wch = np.ascontiguousarray(Wc[2 * hid :].astype(np.float16))
    Wf = np.asarray(W_f, dtype=np.float32)
    wfd = np.ascontiguousarray(
        Wf.reshape(2, hid, hid).transpose(1, 0, 2).astype(np.float16)
    ).reshape(hid, 2 * hid)
    ufs = np.ascontiguousarray(np.asarray(U_f, dtype=np.float32).astype(np.float16))
    bc3 = np.ascontiguousarray(
        np.asarray(b_combined, dtype=np.float32).reshape(3, hid).T
    )
    bf1 = np.ascontiguousarray(np.asarray(b_f, dtype=np.float32).reshape(hid, 1))
    return wcx, wch, wfd, ufs, bc3, bf1


def kernel(
    inputs,
    child_c,
    child_h,
    segment_ids,
    W_combined,
    b_combined,
    W_f,
    U_f,
    b_f,
):
    inputs = np.asarray(inputs, dtype=np.float32)
    child_c = np.asarray(child_c, dtype=np.float32)
    child_h = np.asarray(child_h, dtype=np.float32)
    n, in_dim = inputs.shape
    hid = U_f.shape[0]

    child_c, child_h, cpn = _pad_children(child_c, child_h, segment_ids, n)

    assert n % N_CORES == 0
    npc = n // N_CORES
    npp = ((npc + 511) // 512) * 512  # padded nodes per core

    zero_bias = not (np.any(b_combined) or np.any(b_f))
    nc = _get_program(npp, in_dim, hid, cpn, zero_bias)
    wcx, wch, wfd, ufs, bc3, bf1 = _stage_weights(
        W_combined, W_f, U_f, b_combined, b_f, hid
    )

    in_maps = []
    for c in range(N_CORES):
        n0, n1 = c * npc, (c + 1) * npc
        e0, e1 = n0 * cpn, n1 * cpn
        xpad = np.zeros((hid, 2, npp), np.float16)
        xpad[:, :, :npc] = inputs[n0:n1].reshape(npc, 2, hid).transpose(2, 1, 0)
        chpad = np.zeros((hid, cpn, npp), np.float16)
        chpad[:, :, :npc] = (
            child_h[e0:e1].reshape(npc, cpn, hid).transpose(2, 1, 0)
        )
        ccpad = np.zeros((hid, cpn, npp), np.float16)
        ccpad[:, :, :npc] = (
            child_c[e0:e1].reshape(npc, cpn, hid).transpose(2, 1, 0)
        )
        in_maps.append(
            {
                "xT": xpad.reshape(hid, 2 * npp),
                "ch": chpad.reshape(hid, cpn * npp),
                "cc": ccpad.reshape(hid, cpn * npp),
                "wcx": wcx,
                "wch": wch,
                "wfd": wfd,
                "uf": ufs,
                "bc3": bc3,
                "bf1": bf1,
            }
        )

    res = run_bass_kernel_spmd(
        nc, in_maps, core_ids=list(range(N_CORES)), trace=TRACE
    )
    global LAST_RESULTS
    LAST_RESULTS = res

    c_full = np.empty((n, hid), np.float32)
    h_full = np.empty((n, hid), np.float32)
    for c in range(N_CORES):
        n0, n1 = c * npc, (c + 1) * npc
        c_full[n0:n1] = res.results[c]["cT"][:, :npc].T.astype(np.float32)
        h_full[n0:n1] = res.results[c]["hT"][:, :npc].T.astype(np.float32)
    return (c_full, h_full)


# revision 4
# speedup vs baseline: 1.0182x; 1.0182x over previous
"""Batch Child-Sum TreeLSTM cell on 8 Trainium2 NeuronCores.

Strategy (data-parallel over nodes; fp16 matmuls, PE-roofline ~195us/core):
  - Shard the N nodes (and their contiguous child segments) evenly across the
    8 cores; replicate the small weight matrices. Irregular sorted
    segment_ids are first regularized host-side by zero-padding every node to
    max_children slots (exact: padded slots contribute 0).
  - Host stages activations feature-major (features on SBUF partitions) and
    child-major (one contiguous slab per child slot), cast to fp16 (same PE
    rate as bf16, 10-bit mantissa - strictly more accurate for this O(5)
    data; fp8 was measured too coarse: ~3e-2 rel err vs the 2e-2 gate).
  - PE: 18 matmul streams per node (z: 3 chunks x (2 x-halves + h_tilde);
    f: 3 slots x (U_f h_k + 2 W_f-halves)), all [128,256]-wide into PSUM.
  - PSUM: per 256-node sub, one 5-region tile [z_i|z_o|f0|f1|f2] plus a
    separate z_u tile, both double-buffered (6+2 half-banks of 8) so the PE
    never waits on ACT evacuation (the old kernel single-buffered the f
    PSUM, serializing PE behind ACT every sub-tile).
  - ACT (the old second bottleneck at ~192us) drops to ~176us: ONE fused
    sigmoid evacuates z_i|z_o|f0|f1|f2 per sub (biases are zero per the
    input spec; a general per-region-bias path is kept for nonzero biases),
    tanh(z_u) per sub, tanh(c) once per 2560-node macro.
  - DVE (old ~181us) drops to ~140us: the whole gate chain runs ONCE per
    macro as 9 wide fp16 2x-mode tensor ops (h_tilde child-sum, f*c, segment
    sums, c/h assembly) using strided 3D access patterns over the per-sub
    sigmoid outputs - per-op overhead (~100 cyc) amortizes over 2560 cols.
  - Outputs store as fp16 on the idle gpsimd SWDGE queue; host upcasts.
    Engine budgets/core: PE ~195us (bound), DMA 64MB ~170us, ACT ~176us,
    DVE ~140us.
"""

from contextlib import ExitStack

import numpy as np

import concourse.bass as bass
import concourse.bacc as bacc
import concourse.tile as tile
from concourse import mybir
from concourse.bass_utils import run_bass_kernel_spmd

F32 = mybir.dt.float32
FP16 = mybir.dt.float16

N_CORES = 8

# Tiling (in nodes). SUB: PSUM region width (5 regions x 256 f32 = 2.5 banks
# + z_u half-bank, double-buffered). MACRO: DMA / SBUF / DVE-chain
# granularity.
SUB = 256
MACRO = 2560


def _chunks(total, step):
    out = []
    off = 0
    while off < total:
        out.append((off, min(step, total - off)))
        off += step
    return out


def build_program(npc, in_dim, hid, cpn, zero_bias=True):
    """Bass program for one core's shard: npc nodes, npc*cpn edges."""
    assert in_dim == 256 and hid == 128
    assert npc % 512 == 0
    assert 1 <= cpn <= 4, "PSUM layout fits at most 4 child slots"
    NR = 2 + cpn  # A-tile regions per sub: si, so, f0..f{cpn-1}
    NZW = NR * SUB
    FOFF = 2 * SUB  # f-region offset inside A

    nc = bacc.Bacc("TRN2", target_bir_lowering=False, debug=False)

    xT = nc.dram_tensor("xT", [hid, 2 * npc], FP16, kind="ExternalInput").ap()
    ch = nc.dram_tensor("ch", [hid, cpn * npc], FP16, kind="ExternalInput").ap()
    cc = nc.dram_tensor("cc", [hid, cpn * npc], FP16, kind="ExternalInput").ap()
    wcx = nc.dram_tensor("wcx", [hid, 2 * 3 * hid], FP16, kind="ExternalInput").ap()
    wch = nc.dram_tensor("wch", [hid, 3 * hid], FP16, kind="ExternalInput").ap()
    wfd = nc.dram_tensor("wfd", [hid, 2 * hid], FP16, kind="ExternalInput").ap()
    uf = nc.dram_tensor("uf", [hid, hid], FP16, kind="ExternalInput").ap()
    bc3 = nc.dram_tensor("bc3", [hid, 3], F32, kind="ExternalInput").ap()
    bf1 = nc.dram_tensor("bf1", [hid, 1], F32, kind="ExternalInput").ap()

    cT = nc.dram_tensor("cT", [hid, npc], FP16, kind="ExternalOutput").ap()
    hT = nc.dram_tensor("hT", [hid, npc], FP16, kind="ExternalOutput").ap()

    xT3 = xT.rearrange("p (i n) -> p i n", i=2)
    ch3 = ch.rearrange("p (c n) -> p c n", c=cpn)
    cc3 = cc.rearrange("p (c n) -> p c n", c=cpn)

    ACTF = mybir.ActivationFunctionType

    with tile.TileContext(nc) as tc, ExitStack() as ctx:
        consts = ctx.enter_context(tc.tile_pool(name="consts", bufs=1))
        macro_pool = ctx.enter_context(tc.tile_pool(name="macro", bufs=2))
        tail_pool = ctx.enter_context(tc.tile_pool(name="tail", bufs=1))
        psum = ctx.enter_context(tc.tile_pool(name="psum", bufs=2, space="PSUM"))

        # ---- weights (resident) ----
        wcx_sb = consts.tile([128, 2 * 3 * hid], FP16, tag="wcx")
        nc.sync.dma_start(out=wcx_sb, in_=wcx)
        wcx3 = wcx_sb.rearrange("p (i m) -> p i m", i=2)
        wch_sb = consts.tile([128, 3 * hid], FP16, tag="wch")
        nc.sync.dma_start(out=wch_sb, in_=wch)
        wfd_sb = consts.tile([128, 2 * hid], FP16, tag="wfd")
        nc.sync.dma_start(out=wfd_sb, in_=wfd)
        wfd3 = wfd_sb.rearrange("p (i m) -> p i m", i=2)
        uf_sb = consts.tile([128, hid], FP16, tag="uf")
        nc.sync.dma_start(out=uf_sb, in_=uf)
        bc_sb = consts.tile([128, 3], F32, tag="bc3")
        nc.sync.dma_start(out=bc_sb, in_=bc3)
        bf_sb = consts.tile([128, 1], F32, tag="bf1")
        nc.sync.dma_start(out=bf_sb, in_=bf1)

        macro_plan = []
        if npc > 2 * MACRO:
            # small first macro (compute starts early); tail tapers so the
            # final gate chain + stores drain quickly
            macro_plan.append((0, 512))
            macro_plan += [(512 + o, s) for o, s in _chunks(npc - 512 - 768, MACRO)]
            macro_plan.append((npc - 768, 512))
            macro_plan.append((npc - 256, 256))
        else:
            macro_plan = _chunks(npc, MACRO)

        def issue_loads(m0, msz):
            """Input DMAs + the h_tilde child-sum for one macro. Issued one
            macro AHEAD of use so the PE never waits on the DVE-computed
            h_tilde behind the previous macro's gate chain in the in-order
            DVE queue."""
            x_t = macro_pool.tile([128, 2 * msz], FP16, tag="x")
            nc.sync.dma_start(
                out=x_t.rearrange("p (i n) -> p i n", i=2),
                in_=xT3[:, :, m0 : m0 + msz],
            )
            ch_t = macro_pool.tile([128, cpn * msz], FP16, tag="ch")
            nc.sync.dma_start(
                out=ch_t.rearrange("p (c n) -> p c n", c=cpn),
                in_=ch3[:, :, m0 : m0 + msz],
            )
            cht3 = ch_t.rearrange("p (c n) -> p c n", c=cpn)
            cc_t = macro_pool.tile([128, cpn * msz], FP16, tag="cc")
            nc.sync.dma_start(
                out=cc_t.rearrange("p (c n) -> p c n", c=cpn),
                in_=cc3[:, :, m0 : m0 + msz],
            )
            # h_tilde for the whole macro: 2 wide DVE adds (PE consumes it)
            ht_t = macro_pool.tile([128, msz], FP16, tag="ht")
            nc.vector.tensor_add(ht_t, cht3[:, 0, :], cht3[:, 1, :])
            for ci in range(2, cpn):
                nc.vector.tensor_add(ht_t, ht_t, cht3[:, ci, :])
            return x_t, ch_t, cc_t, ht_t

        staged = {0: issue_loads(*macro_plan[0])}
        for idx, (m0, msz) in enumerate(macro_plan):
            if idx + 1 < len(macro_plan):
                staged[idx + 1] = issue_loads(*macro_plan[idx + 1])
            x_t, ch_t, cc_t, ht_t = staged.pop(idx)
            xt3 = x_t.rearrange("p (i n) -> p i n", i=2)
            cht3 = ch_t.rearrange("p (c n) -> p c n", c=cpn)
            cct3 = cc_t.rearrange("p (c n) -> p c n", c=cpn)
            nsub = msz // SUB
            sio_t = macro_pool.tile([128, nsub * NZW], FP16, tag="sio")
            tu_t = macro_pool.tile([128, msz], FP16, tag="tu")

            for k in range(nsub):
                s0 = k * SUB
                A = psum.tile([128, NZW], F32, tag="A")
                zu = psum.tile([128, SUB], F32, tag="zu")
                xs = xt3[:, :, s0 : s0 + SUB]
                hts = ht_t[:, s0 : s0 + SUB]
                # z chunks: j=0 -> si, j=1 -> so, j=2 -> zu tile
                for j in range(3):
                    out = zu if j == 2 else A[:, j * SUB : (j + 1) * SUB]
                    for i in range(2):
                        nc.tensor.matmul(
                            out,
                            lhsT=wcx3[:, i, 128 * j : 128 * (j + 1)],
                            rhs=xs[:, i, :],
                            start=(i == 0),
                            stop=False,
                        )
                    nc.tensor.matmul(
                        out,
                        lhsT=wch_sb[:, 128 * j : 128 * (j + 1)],
                        rhs=hts,
                        start=False,
                        stop=True,
                    )
                # forget gates, child-major: f_c = U_f h_c + W_f x
                for c in range(cpn):
                    out = A[:, FOFF + c * SUB : FOFF + (c + 1) * SUB]
                    nc.tensor.matmul(
                        out,
                        lhsT=uf_sb,
                        rhs=cht3[:, c, s0 : s0 + SUB],
                        start=True,
                        stop=False,
                    )
                    for i in range(2):
                        nc.tensor.matmul(
                            out, lhsT=wfd3[:, i, :], rhs=xs[:, i, :],
                            start=False, stop=(i == 1),
                        )

                sio = sio_t[:, k * NZW : (k + 1) * NZW]
                if zero_bias:
                    # one transcendental pass for all five A regions
                    nc.scalar.activation(sio, A, ACTF.Sigmoid)
                else:
                    nc.scalar.activation(
                        sio[:, 0:SUB], A[:, 0:SUB], ACTF.Sigmoid,
                        bias=bc_sb[:, 0:1],
                    )
                    nc.scalar.activation(
                        sio[:, SUB : 2 * SUB], A[:, SUB : 2 * SUB], ACTF.Sigmoid,
                        bias=bc_sb[:, 1:2],
                    )
                    nc.scalar.activation(
                        sio[:, FOFF:NZW], A[:, FOFF:NZW], ACTF.Sigmoid,
                        bias=bf_sb[:, 0:1],
                    )
                nc.scalar.activation(
                    tu_t[:, s0 : s0 + SUB], zu, ACTF.Tanh,
                    **({} if zero_bias else {"bias": bc_sb[:, 2:3]}),
                )

            # ---- whole-macro gate chain on DVE (wide 2x-mode fp16 ops) ----
            sioM = sio_t.rearrange("p (k w) -> p k w", w=NZW)
            si3 = sioM[:, :, 0:SUB]
            so3 = sioM[:, :, SUB : 2 * SUB]
            c_t = macro_pool.tile([128, msz], FP16, tag="c_out")
            h_t = macro_pool.tile([128, msz], FP16, tag="h_out")
            tc_t = macro_pool.tile([128, msz], FP16, tag="tanh_c")
            fjc_t = tail_pool.tile([128, cpn * msz], FP16, tag="fjc")
            fjc3 = fjc_t.rearrange("p (c n) -> p c n", c=cpn)
            for c in range(cpn):
                fcol = sioM[:, :, FOFF + c * SUB : FOFF + (c + 1) * SUB]
                nc.vector.tensor_mul(
                    fjc3[:, c, :].rearrange("p (k n) -> p k n", n=SUB),
                    fcol,
                    cct3[:, c, :].rearrange("p (k n) -> p k n", n=SUB),
                )
            if cpn == 1:
                fc_t = fjc_t
            else:
                fc_t = tail_pool.tile([128, msz], FP16, tag="fc")
                nc.vector.tensor_add(fc_t, fjc3[:, 0, :], fjc3[:, 1, :])
                for ci in range(2, cpn):
                    nc.vector.tensor_add(fc_t, fc_t, fjc3[:, ci, :])
            c3 = c_t.rearrange("p (k n) -> p k n", n=SUB)
            nc.vector.tensor_mul(c3, si3, tu_t.rearrange("p (k n) -> p k n", n=SUB))
            nc.vector.tensor_add(c_t, c_t, fc_t)
            nc.scalar.activation(tc_t, c_t, ACTF.Tanh)
            nc.vector.tensor_mul(
                h_t.rearrange("p (k n) -> p k n", n=SUB),
                so3,
                tc_t.rearrange("p (k n) -> p k n", n=SUB),
            )
            # stores ride the otherwise-idle gpsimd SWDGE queue so the next
            # macro's loads never queue behind them in the SP HWDGE FIFO
            nc.gpsimd.dma_start(out=cT[:, m0 : m0 + msz], in_=c_t)
            nc.gpsimd.dma_start(out=hT[:, m0 : m0 + msz], in_=h_t)

    nc.compile()
    return nc


TRACE = False  # set True (e.g. from test.py) to capture an NTFF profile
LAST_RESULTS = None  # BassKernelResults of the most recent kernel() call

_PROGRAM_CACHE = {}


def _get_program(npc, in_dim, hid, cpn, zero_bias):
    key = (npc, in_dim, hid, cpn, zero_bias, SUB, MACRO)
    if key not in _PROGRAM_CACHE:
        _PROGRAM_CACHE[key] = build_program(npc, in_dim, hid, cpn, zero_bias)
    return _PROGRAM_CACHE[key]


def _pad_children(child_c, child_h, segment_ids, n):
    """Regularize to exactly max_c children per node (zero padding is exact:
    padded slots contribute sigmoid(..)*0 to fc and 0 to the child sum)."""
    seg = np.asarray(segment_ids).astype(np.int64)
    e = seg.shape[0]
    counts = np.bincount(seg, minlength=n)
    max_c = int(counts.max()) if e else 1
    if e == n * max_c and np.all(counts == max_c):
        return child_c, child_h, max_c  # already regular (and sorted)
    hid = child_h.shape[1]
    slot = np.arange(e, dtype=np.int64) - np.repeat(
        np.concatenate([[0], np.cumsum(counts)[:-1]]), counts
    )
    cc = np.zeros((n * max_c, hid), np.float32)
    ch = np.zeros((n * max_c, hid), np.float32)
    idx = seg * max_c + slot
    cc[idx] = child_c
    ch[idx] = child_h
    return cc, ch, max_c


def _stage_weights(W_combined, W_f, U_f, b_combined, b_f, hid):
    Wc = np.asarray(W_combined, dtype=np.float32)
    wcx = np.ascontiguousarray(
        Wc[: 2 * hid].reshape(2, hid, 3 * hid).transpose(1, 0, 2).astype(np.float16)
    ).reshape(hid, 2 * 3 * hid)
    wch = np.ascontiguousarray(Wc[2 * hid :].astype(np.float16))
    Wf = np.asarray(W_f, dtype=np.float32)
    wfd = np.ascontiguousarray(
        Wf.reshape(2, hid, hid).transpose(1, 0, 2).astype(np.float16)
    ).reshape(hid, 2 * hid)
    ufs = np.ascontiguousarray(np.asarray(U_f, dtype=np.float32).astype(np.float16))
    bc3 = np.ascontiguousarray(
        np.asarray(b_combined, dtype=np.float32).reshape(3, hid).T
    )
    bf1 = np.ascontiguousarray(np.asarray(b_f, dtype=np.float32).reshape(hid, 1))
    return wcx, wch, wfd, ufs, bc3, bf1


def kernel(
    inputs,
    child_c,
    child_h,
    segment_ids,
    W_combined,
    b_combined,
    W_f,
    U_f,
    b_f,
):
    inputs = np.asarray(inputs, dtype=np.float32)
    child_c = np.asarray(child_c, dtype=np.float32)
    child_h = np.asarray(child_h, dtype=np.float32)
    n, in_dim = inputs.shape
    hid = U_f.shape[0]

    child_c, child_h, cpn = _pad_children(child_c, child_h, segment_ids, n)

    assert n % N_CORES == 0
    npc = n // N_CORES
    npp = ((npc + 511) // 512) * 512  # padded nodes per core

    zero_bias = not (np.any(b_combined) or np.any(b_f))
    nc = _get_program(npp, in_dim, hid, cpn, zero_bias)
    wcx, wch, wfd, ufs, bc3, bf1 = _stage_weights(
        W_combined, W_f, U_f, b_combined, b_f, hid
    )

    in_maps = []
    for c in range(N_CORES):
        n0, n1 = c * npc, (c + 1) * npc
        e0, e1 = n0 * cpn, n1 * cpn
        xpad = np.zeros((hid, 2, npp), np.float16)
        xpad[:, :, :npc] = inputs[n0:n1].reshape(npc, 2, hid).transpose(2, 1, 0)
        chpad = np.zeros((hid, cpn, npp), np.float16)
        chpad[:, :, :npc] = (
            child_h[e0:e1].reshape(npc, cpn, hid).transpose(2, 1, 0)
        )
        ccpad = np.zeros((hid, cpn, npp), np.float16)
        ccpad[:, :, :npc] = (
            child_c[e0:e1].reshape(npc, cpn, hid).transpose(2, 1, 0)
        )
        in_maps.append(
            {
                "xT": xpad.reshape(hid, 2 * npp),
                "ch": chpad.reshape(hid, cpn * npp),
                "cc": ccpad.reshape(hid, cpn * npp),
                "wcx": wcx,
                "wch": wch,
                "wfd": wfd,
                "uf": ufs,
                "bc3": bc3,
                "bf1": bf1,
            }
        )

    res = run_bass_kernel_spmd(
        nc, in_maps, core_ids=list(range(N_CORES)), trace=TRACE
    )
    global LAST_RESULTS
    LAST_RESULTS = res

    c_full = np.empty((n, hid), np.float32)
    h_full = np.empty((n, hid), np.float32)
    for c in range(N_CORES):
        n0, n1 = c * npc, (c + 1) * npc
        c_full[n0:n1] = res.results[c]["cT"][:, :npc].T.astype(np.float32)
        h_full[n0:n1] = res.results[c]["hT"][:, :npc].T.astype(np.float32)
    return (c_full, h_full)
